# revision 11
# baseline (speedup 1.0000x reference)
"""Trainium2 Bass kernel for a 3x3 'same' conv: x [8,16,512,512] f32, weight [16,144].

Strategy (data-parallel over batch, 1 image per NeuronCore, 8 cores):
  - Row-group the image into groups of R=6 output rows. A group loads J=8 input
    rows (R + 2 halo) in one DMA: SBUF x-tile partition p = ci*J + j holds input
    row Y+j of channel ci, where Y = clamp(y0-1, 0, H-J) -> K = 128 contraction
    partitions.
  - Stationary weights per (kw tap, boundary variant b): [128, 96] matrices
    W[(ci*J+j), (co*R+r)] = w[co, ci, kh, kw] at j = r + kh + (b-1), entries
    with j outside [0, J) dropped (those are the zero-pad rows). b=0 for the
    top group, b=2 for the bottom group, b=1 interior.
  - Three accumulating matmuls (kw = 0, 1, 2) into one PSUM bank [96, 512].
    The x-tile is [128, 514] with zeroed pad columns 0 and 513 and data in
    [1, 513); matmul kw reads rhs columns [kw, kw+512) so the +-1 horizontal
    taps become SBUF-side column offsets, keeping every PSUM write a full
    aligned bank (fp32r dst-pattern ISA restriction).
  - PSUM -> SBUF via VectorE copy, then DMA to HBM.
  - float32r matmul dtype: full-rate fp32 on the PE (4x faster than float32).
"""

import os
from contextlib import ExitStack

import numpy as np

C_OUT, C_IN, KH, KW = 16, 16, 3, 3
H = W = 512
B = 8
R = 6           # output rows per group
J = R + 2       # input rows per group
M = C_OUT * R   # 96 psum partitions
K = C_IN * J    # 128 contraction partitions
NV = KW * 3     # stationary variants: 3 kw taps x 3 boundary kinds
GROUP_Y0 = [6 * g for g in range(85)] + [506]

# matmul dtype: "float32r" (full-rate, relaxed precision) or "float32" (exact, 1/4 rate)
MM_DTYPE_NAME = os.environ.get("CONV_MM_DTYPE", "float32r")

_CACHE = {}


def _build_weights(weight: np.ndarray) -> np.ndarray:
    """[16,144] -> [128, 9*96] stationary matrices, variant v = kw*3 + b.

    psum[co*R+r, xo] += sum_{ci,j} wk[ci*J+j, v, co*R+r] * x[ci, Y+j, xo+kw-1]
    with Y = y0-1+(1-b)... i.e. row Y+j = y0+r+kh-1 exactly when j = r+kh+(b-1).
    """
    w = np.asarray(weight, dtype=np.float32).reshape(C_OUT, C_IN, KH, KW)
    wk = np.zeros((KW, 3, K, M), np.float32)
    for kw in range(KW):
        for b in range(3):
            for co in range(C_OUT):
                for r in range(R):
                    for kh in range(KH):
                        j = r + kh + (b - 1)
                        if 0 <= j < J:
                            for ci in range(C_IN):
                                wk[kw, b, ci * J + j, co * R + r] = w[co, ci, kh, kw]
    # -> [K, kw, b, M] -> [K, 9*M]
    return np.ascontiguousarray(wk.transpose(2, 0, 1, 3).reshape(K, NV * M))


def _build_nc():
    import concourse.tile as tile
    from concourse import bacc, mybir

    mm_dt = getattr(mybir.dt, MM_DTYPE_NAME)
    f32 = mybir.dt.float32

    nc = bacc.Bacc(
        "TRN2", target_bir_lowering=False, debug=False,
        enable_asserts=False, num_devices=B,
    )
    # Inputs are declared with the matmul dtype end-to-end (float32r has the
    # same 4-byte layout as float32) so the BIR verifier sees a consistent
    # fp32r producer->consumer chain.
    x = nc.dram_tensor("x", [C_IN, H, W], mm_dt, kind="ExternalInput").ap()
    wkin = nc.dram_tensor("wk", [K, NV * M], mm_dt, kind="ExternalInput").ap()
    out = nc.dram_tensor("out", [C_OUT, H, W], f32, kind="ExternalOutput").ap()

    with tile.TileContext(nc) as tc, ExitStack() as ctx:
        wpool = ctx.enter_context(tc.tile_pool(name="wpool", bufs=1))
        xpool = ctx.enter_context(tc.tile_pool(name="xpool", bufs=6))
        opool = ctx.enter_context(tc.tile_pool(name="opool", bufs=6))
        ppool = ctx.enter_context(tc.tile_pool(name="ppool", bufs=8, space="PSUM"))

        wt = wpool.tile([K, NV * M], mm_dt, name="wt")
        nc.sync.dma_start(out=wt[:], in_=wkin[:])

        def wap(kw, b):
            v = kw * 3 + b
            return wt[:, v * M : (v + 1) * M]

        for y0 in GROUP_Y0:
            b = 0 if y0 == 0 else (2 if y0 == H - R else 1)
            Y = min(max(y0 - 1, 0), H - J)

            xtile = xpool.tile([K, W + 2], mm_dt, name="xtile", tag="xtile")
            nc.vector.memset(xtile[:, 0:1].bitcast(f32), 0.0)
            nc.vector.memset(xtile[:, W + 1 : W + 2].bitcast(f32), 0.0)
            nc.sync.dma_start(out=xtile[:, 1 : W + 1], in_=x[:, Y : Y + J, :])

            pt = ppool.tile([M, W], f32, name="pt", tag="pt")
            for kw in range(KW):
                nc.tensor.matmul(pt[:, 0:W], wap(kw, b), xtile[:, kw : kw + W],
                                 start=(kw == 0), stop=(kw == KW - 1))

            ot = opool.tile([M, W], f32, name="ot", tag="ot")
            nc.vector.tensor_copy(ot[:], pt[:])
            nc.sync.dma_start(out=out[:, y0 : y0 + R, :], in_=ot[:])

    nc.compile()
    return nc


def get_nc():
    if "nc" not in _CACHE:
        _CACHE["nc"] = _build_nc()
    return _CACHE["nc"]


def run(x: np.ndarray, weight: np.ndarray, **spmd_kwargs):
    """Run the conv on 8 cores; returns (out [8,16,512,512], BassKernelResults)."""
    from concourse.bass_utils import run_bass_kernel_spmd

    x = np.ascontiguousarray(np.asarray(x, dtype=np.float32))
    wk = _build_weights(weight)
    nc = get_nc()
    in_maps = [{"x": x[b], "wk": wk} for b in range(B)]
    res = run_bass_kernel_spmd(nc, in_maps, list(range(B)), **spmd_kwargs)
    out = np.stack([res.results[b]["out"] for b in range(B)], axis=0)
    return out, res


def kernel(x: np.ndarray, weight: np.ndarray) -> np.ndarray:
    return run(x, weight)[0]


# revision 13
# speedup vs baseline: 1.0735x; 1.0735x over previous
"""Trainium2 Bass kernel for a 3x3 'same' conv: x [8,16,512,512] f32, weight [16,144].

Strategy (data-parallel over batch, 1 image per NeuronCore, 8 cores):
  - Row-group the image into groups of R=6 output rows. A group loads J=8 input
    rows (R + 2 halo) in one DMA: SBUF x-tile partition p = ci*J + j holds input
    row Y+j of channel ci, where Y = clamp(y0-1, 0, H-J) -> K = 128 contraction
    partitions.
  - Stationary weights per (kw tap, boundary variant b): [128, 96] matrices
    W[(ci*J+j), (co*R+r)] = w[co, ci, kh, kw] at j = r + kh + (b-1), entries
    with j outside [0, J) dropped (those are the zero-pad rows). b=0 for the
    top group, b=2 for the bottom group, b=1 interior.
  - Three accumulating matmuls (kw = 0, 1, 2) into one PSUM bank [96, 512].
    The x-tile is [128, 514] with zeroed pad columns 0 and 513 and data in
    [1, 513); matmul kw reads rhs columns [kw, kw+512) so the +-1 horizontal
    taps become SBUF-side column offsets, keeping every PSUM write a full
    aligned bank (fp32r dst-pattern ISA restriction).
  - PSUM -> SBUF via VectorE copy, then DMA to HBM.
  - float32r matmul dtype: full-rate fp32 on the PE (4x faster than float32).
"""

import os
from contextlib import ExitStack

import numpy as np

C_OUT, C_IN, KH, KW = 16, 16, 3, 3
H = W = 512
B = 8
R = 6           # output rows per group
J = R + 2       # input rows per group
M = C_OUT * R   # 96 psum partitions
K = C_IN * J    # 128 contraction partitions
NV = KW * 3     # stationary variants: 3 kw taps x 3 boundary kinds
GROUP_Y0 = [6 * g for g in range(85)] + [506]

# matmul dtype: "float32r" (full-rate, relaxed precision) or "float32" (exact, 1/4 rate)
MM_DTYPE_NAME = os.environ.get("CONV_MM_DTYPE", "float32r")

_CACHE = {}


def _build_weights(weight: np.ndarray) -> np.ndarray:
    """[16,144] -> [128, 9*96] stationary matrices, variant v = kw*3 + b.

    psum[co*R+r, xo] += sum_{ci,j} wk[ci*J+j, v, co*R+r] * x[ci, Y+j, xo+kw-1]
    with Y = y0-1+(1-b)... i.e. row Y+j = y0+r+kh-1 exactly when j = r+kh+(b-1).
    """
    w = np.asarray(weight, dtype=np.float32).reshape(C_OUT, C_IN, KH, KW)
    wk = np.zeros((KW, 3, K, M), np.float32)
    for kw in range(KW):
        for b in range(3):
            for co in range(C_OUT):
                for r in range(R):
                    for kh in range(KH):
                        j = r + kh + (b - 1)
                        if 0 <= j < J:
                            for ci in range(C_IN):
                                wk[kw, b, ci * J + j, co * R + r] = w[co, ci, kh, kw]
    # -> [K, kw, b, M] -> [K, 9*M]
    return np.ascontiguousarray(wk.transpose(2, 0, 1, 3).reshape(K, NV * M))


def _build_nc():
    import concourse.tile as tile
    from concourse import bacc, mybir

    mm_dt = getattr(mybir.dt, MM_DTYPE_NAME)
    f32 = mybir.dt.float32

    nc = bacc.Bacc(
        "TRN2", target_bir_lowering=False, debug=False,
        enable_asserts=False, num_devices=B,
    )
    # Inputs are declared with the matmul dtype end-to-end (float32r has the
    # same 4-byte layout as float32) so the BIR verifier sees a consistent
    # fp32r producer->consumer chain.
    x = nc.dram_tensor("x", [C_IN, H, W], mm_dt, kind="ExternalInput").ap()
    wkin = nc.dram_tensor("wk", [K, NV * M], mm_dt, kind="ExternalInput").ap()
    out = nc.dram_tensor("out", [C_OUT, H, W], f32, kind="ExternalOutput").ap()

    with tile.TileContext(nc) as tc, ExitStack() as ctx:
        wpool = ctx.enter_context(tc.tile_pool(name="wpool", bufs=1))
        xpool = ctx.enter_context(tc.tile_pool(name="xpool", bufs=12))
        opool = ctx.enter_context(tc.tile_pool(name="opool", bufs=12))
        ppool = ctx.enter_context(tc.tile_pool(name="ppool", bufs=8, space="PSUM"))

        wt = wpool.tile([K, NV * M], mm_dt, name="wt")
        nc.scalar.dma_start(out=wt[:], in_=wkin[:])

        def wap(kw, b):
            v = kw * 3 + b
            return wt[:, v * M : (v + 1) * M]

        for y0 in GROUP_Y0:
            b = 0 if y0 == 0 else (2 if y0 == H - R else 1)
            Y = min(max(y0 - 1, 0), H - J)

            xtile = xpool.tile([K, W + 2], mm_dt, name="xtile", tag="xtile")
            nc.vector.memset(xtile[:, 0:1].bitcast(f32), 0.0)
            nc.vector.memset(xtile[:, W + 1 : W + 2].bitcast(f32), 0.0)
            eng_in = nc.sync if (y0 // R) % 2 == 0 else nc.scalar
            eng_in.dma_start(out=xtile[:, 1 : W + 1], in_=x[:, Y : Y + J, :])

            pt = ppool.tile([M, W], f32, name="pt", tag="pt")
            for kw in range(KW):
                nc.tensor.matmul(pt[:, 0:W], wap(kw, b), xtile[:, kw : kw + W],
                                 start=(kw == 0), stop=(kw == KW - 1))

            ot = opool.tile([M, W], f32, name="ot", tag="ot")
            nc.vector.tensor_copy(ot[:], pt[:])
            eng_out = nc.scalar if (y0 // R) % 2 == 0 else nc.sync
            eng_out.dma_start(out=out[:, y0 : y0 + R, :], in_=ot[:])

    nc.compile()
    return nc


def get_nc():
    if "nc" not in _CACHE:
        _CACHE["nc"] = _build_nc()
    return _CACHE["nc"]


def run(x: np.ndarray, weight: np.ndarray, **spmd_kwargs):
    """Run the conv on 8 cores; returns (out [8,16,512,512], BassKernelResults)."""
    from concourse.bass_utils import run_bass_kernel_spmd

    x = np.ascontiguousarray(np.asarray(x, dtype=np.float32))
    wk = _build_weights(weight)
    nc = get_nc()
    in_maps = [{"x": x[b], "wk": wk} for b in range(B)]
    res = run_bass_kernel_spmd(nc, in_maps, list(range(B)), **spmd_kwargs)
    out = np.stack([res.results[b]["out"] for b in range(B)], axis=0)
    return out, res


def kernel(x: np.ndarray, weight: np.ndarray) -> np.ndarray:
    return run(x, weight)[0]
